# revision 43
# baseline (speedup 1.0000x reference)
"""Trainium2 Bass kernel v2 for nn_Kernel_12281761695451725822_53472342835843.

Computation (per sample n, channel c):
  t3 = Conv1x5(x, w3)                      (channel-mixing 1x5 conv, pad 2)
  t7 = sum over 9 (oh,ow) terms of w7[c,3*ow+oh] * max(A, B)
  out = t7 * t3

Multi-engine split (TimelineSim-balanced): PE does the t3 conv plus 7 of
the 9 t7 taps (diagonal matmuls, PSUM-accumulated); two taps are offloaded
as ACT per-partition scaled copies + DVE adds; DVE computes the 6 shared
pair-max tensors per 16-row strip (reading odd canvas offsets directly --
no shifted canvas copy) and the final t7*t3 multiply; ACT drains PSUM; the
gpsimd engine issues the casting input DMA (strided fp32->fp16 straight
into the padded canvas), zeroes borders and patches the w=0 wrap column.
Output is written fp16 and upcast on the host. PE warmup matmuls establish
the p-state ramp before the main burst; the input is chunked so the max
pipeline starts early and stays 4 strips ahead.
"""

import numpy as np

N, C, H, W = 16, 64, 128, 128
NCORES = 8
NLOC = N // NCORES          # samples per core
P = 128                     # partitions = NLOC * C
ROWS = H + 2                # canvas rows, storage row = h + 1
COLS = 140                  # canvas cols, storage col = u + UOFF, u in [-9, 131)
UOFF = 9
MS = 16                     # max-strip height (rows per M tile batch)
HS = 8                      # matmul strip height
OS = 16                     # output block rows per DMA
NMS = H // MS
NHS = H // HS
NOB = H // OS

# term i: (oh, ow, (dh, dw) of shared max, read offset (dr, du) into M)
TERMS = [
    (0, 0, (1, 1), -1, -3),
    (0, 1, (1, -1), -1, -1),
    (0, 2, (1, -3), -1, 1),
    (1, 0, (0, 3), 0, -3),
    (1, 1, (0, 1), 0, -1),
    (1, 2, (0, 1), 0, 0),
    (2, 0, (1, -5), 0, 2),
    (2, 1, (1, -3), 0, 2),
    (2, 2, (1, -1), 0, 2),
]
DELTAS = [(1, 1), (1, -1), (1, -3), (0, 3), (0, 1), (1, -5)]
# computed j ranges per delta: M_d[rr, j-j0] = max(x[h, j-3], x[h+dh, j-3+dw])
JR = {
    (1, 1): (0, 130),
    (1, -1): (2, 133),
    (1, -3): (4, 133),
    (0, 3): (0, 130),
    (0, 1): (2, 131),
    (1, -5): (5, 133),
}
# per delta: (row base, row count) of the stored M strip. rr index runs over
# [base, base+rows) where rr = h - ms0 + 1 + dr and dr is per consuming term.
MROW = {
    (1, 1): (0, MS),        # dr = -1 only
    (1, -1): (0, MS + 1),   # dr in {-1, 0}
    (1, -3): (0, MS + 1),
    (0, 3): (1, MS),        # dr = 0 only
    (0, 1): (1, MS),
    (1, -5): (1, MS),
}
# input DMA chunk boundaries in x-row space. First chunk is small so the
# first t3 matmuls can start early; maxes of mstrip m need x rows up to
# ms0+32 (canvas rows [ms0, ms0+33]).
CHUNKS = [(0, 5), (5, 9), (9, 17), (17, 34), (34, 50), (50, 66), (66, 82),
          (82, 98), (98, 114), (114, 128)]
NWARM = 30                  # PE warmup matmuls (p-state ramp establishment)
WROWS = 128                 # rows per warmup matmul
OFFLOAD_TERMS = (0, 3)      # t7 terms computed on ACT+DVE instead of PE
OFFLOAD_MSTRIPS = tuple(range(7))  # mstrips with offloaded terms (last stays
                                   # on PE to keep the tail chain short)


def build_host_weights(w3, w7):
    """Host-side packing of the conv weights into PE lhsT layouts (fp16)."""
    w3 = np.asarray(w3, dtype=np.float32)
    w7 = np.asarray(w7, dtype=np.float32)
    wt3 = np.zeros((5, P, P), dtype=np.float16)
    for k in range(5):
        blk = w3[:, :, 0, k].T.astype(np.float16)  # [ci, co]
        for n in range(NLOC):
            wt3[k, n * C:(n + 1) * C, n * C:(n + 1) * C] = blk
    wt7 = np.zeros((9, P, P), dtype=np.float16)
    w7c = np.zeros((P, 9), dtype=np.float16)
    for i, (oh, ow, _d, _dr, _du) in enumerate(TERMS):
        kidx = 3 * ow + oh
        vals = np.concatenate([w7[:, kidx], w7[:, kidx]]).astype(np.float16)  # [P]
        wt7[i, np.arange(P), np.arange(P)] = vals
        w7c[:, i] = vals
    return (np.ascontiguousarray(wt3.transpose(1, 0, 2)),
            np.ascontiguousarray(wt7.transpose(1, 0, 2)), w7c)


def build_host_canvas(x):
    """Pad + cast the input into the canvas layout on the host."""
    xs = np.asarray(x, dtype=np.float32).reshape(N, C, H, W)
    canv = np.zeros((NCORES, P, ROWS, COLS), dtype=np.float16)
    for core in range(NCORES):
        shard = xs[core * NLOC:(core + 1) * NLOC].reshape(P, H, W)
        canv[core, :, 1:H + 1, UOFF:UOFF + W] = shard.astype(np.float16)
    return canv


def build_program():
    """Build and compile the single-core Bass/Tile program (SPMD-replicated)."""
    import concourse.bacc as bacc
    import concourse.tile as tile
    import concourse.mybir as mybir

    fp16 = mybir.dt.float16
    fp32 = mybir.dt.float32
    AOT = mybir.AluOpType

    nc = bacc.Bacc("TRN2", target_bir_lowering=False, debug=False,
                   enable_asserts=False, num_devices=1)
    canv_d = nc.dram_tensor("canv", [P, ROWS, COLS], fp16, kind="ExternalInput")
    wt3_d = nc.dram_tensor("wt3", [P, 5, P], fp16, kind="ExternalInput")
    wt7_d = nc.dram_tensor("wt7", [P, 9, P], fp16, kind="ExternalInput")
    w7c_d = nc.dram_tensor("w7c", [P, 9], fp16, kind="ExternalInput")
    w7f_d = nc.dram_tensor("w7f", [P, 9], fp32, kind="ExternalInput")
    out_d = nc.dram_tensor("out", [P, H, W], fp16, kind="ExternalOutput")

    with tile.TileContext(nc) as tc:
        with (
            tc.tile_pool(name="persist", bufs=1) as persist,
            tc.tile_pool(name="mpool", bufs=4) as mpool,
            tc.tile_pool(name="ppool", bufs=2, space="PSUM") as ppool,
            tc.tile_pool(name="ppool7", bufs=4, space="PSUM") as ppool7,
            tc.tile_pool(name="s3pool", bufs=4) as s3pool,
            tc.tile_pool(name="s7pool", bufs=3) as s7pool,
            tc.tile_pool(name="opool", bufs=3) as opool,
            tc.tile_pool(name="fixp", bufs=2) as fixp,
            tc.tile_pool(name="tappool", bufs=4) as tappool,
        ):
            canvas = persist.tile([P, ROWS, COLS], fp16, tag="canvas")
            wtile = persist.tile([P, P], fp16, tag="warm")
            wt3_s = persist.tile([P, 5, P], fp16, tag="wt3")
            wt7_s = persist.tile([P, 9, P], fp16, tag="wt7")
            w7c_s = persist.tile([P, 9], fp16, tag="w7c")
            w7f_s = persist.tile([P, 9], fp32, tag="w7f")
            t7c0 = persist.tile([P, H], fp16, tag="t7c0")

            # warmup source: ready at ~0.3us, no DMA dependency
            nc.vector.memset(wtile, 0.25)

            # single SP HWDGE queue, explicit issue order: the two chunks
            # gating t3 strip 0 / first maxes, then weights, then fine-
            # grained bulk chunks (short DMA_ENGINES holds so out-DMAs
            # interleave without queueing behind long input transfers)
            def chunk_dma(ci):
                r0, r1 = CHUNKS[ci]
                c0 = 0 if ci == 0 else 1 + r0
                c1 = ROWS if ci == len(CHUNKS) - 1 else 1 + r1
                nc.sync.dma_start(out=canvas[:, c0:c1, :],
                                  in_=canv_d.ap()[:, c0:c1, :])

            chunk_dma(0)
            chunk_dma(1)
            nc.sync.dma_start(out=wt3_s, in_=wt3_d.ap())
            nc.sync.dma_start(out=wt7_s, in_=wt7_d.ap())
            nc.sync.dma_start(out=w7c_s, in_=w7c_d.ap())
            nc.sync.dma_start(out=w7f_s, in_=w7f_d.ap())
            for ci in range(2, len(CHUNKS)):
                chunk_dma(ci)

            def emit_maxes(m, split=False):
                """split=True: emit each max as two row-halves so the first
                half only depends on the first input chunk (earlier DVE
                start for mstrip 0)."""
                ms0 = m * MS
                mts = {}
                for di, (dh, dw) in enumerate(DELTAS):
                    j0, j1 = JR[(dh, dw)]
                    wd = j1 - j0
                    wpad = wd + (wd & 1)
                    base, rows = MROW[(dh, dw)]
                    mt = mpool.tile([P, rows, wpad], fp16, tag=f"m{di}")
                    pieces = [(0, rows)]
                    if split:
                        c1, c2 = 6 - base - dh, 10 - base - dh
                        pieces = [(0, c1), (c1, c2), (c2, rows)]
                    for (p0, p1) in pieces:
                        in0 = canvas[:, ms0 + base + p0:ms0 + base + p1,
                                     j0 + 6:j0 + 6 + wd]
                        in1 = canvas[:, ms0 + base + dh + p0:
                                     ms0 + base + dh + p1,
                                     j0 + 6 + dw:j0 + 6 + dw + wd]
                        nc.vector.tensor_tensor(mt[:, p0:p1, 0:wd],
                                                in0, in1, AOT.max)
                    mts[(dh, dw)] = mt
                return mts

            def emit_t3(s):
                """t3 matmuls for hstrip s, drain into t3sb block slice."""
                h0 = s * HS
                t3p = ppool.tile([P, HS * W], fp32, tag="t3p")
                for k in range(5):
                    for half in range(2):
                        rhs = canvas[:, 1 + h0 + 4 * half:1 + h0 + 4 * half + 4,
                                     k + 7:k + 7 + W]
                        nc.tensor.matmul(
                            out=t3p[:, 512 * half:512 * half + 512],
                            lhsT=wt3_s[:, k, :], rhs=rhs,
                            start=(k == 0), stop=(k == 4))
                b = h0 // OS
                sub = (h0 % OS) // HS
                if sub == 0:
                    t3sb = s3pool.tile([P, OS, W], fp16, tag="t3sb")
                    emit_t3.cur = t3sb
                t3sb = emit_t3.cur
                nc.scalar.copy(
                    out=t3sb[:, sub * HS:(sub + 1) * HS, :].rearrange(
                        "p a b -> p (a b)"),
                    in_=t3p)
                if sub == 1:
                    emit_t3.blocks[b] = t3sb

            emit_t3.blocks = {}

            def emit_t7(s, mts, skip=()):
                h0 = s * HS
                ms0 = (h0 // MS) * MS
                kept = [i for i in range(len(TERMS)) if i not in skip]
                b = h0 // OS
                sub = (h0 % OS) // HS
                if sub == 0:
                    t7sb = s7pool.tile([P, OS, W], fp16, tag="t7sb")
                    emit_t7.cur = t7sb
                t7sb = emit_t7.cur
                # single-bank PSUM tiles per 4-row half: 4-deep rotation
                # decouples the PE from ACT drain latency
                for half in range(2):
                    t7p = ppool7.tile([P, 512], fp32, tag="t7p")
                    for i, (_oh, _ow, d, dr, du) in enumerate(TERMS):
                        if i in skip:
                            continue
                        mt = mts[d]
                        j0, _j1 = JR[d]
                        base, _rows = MROW[d]
                        rr0 = h0 - ms0 + 1 + dr - base
                        jj0 = du + 3 - j0
                        rhs = mt[:, rr0 + 4 * half:rr0 + 4 * half + 4,
                                 jj0:jj0 + W]
                        nc.tensor.matmul(
                            out=t7p,
                            lhsT=wt7_s[:, i, :], rhs=rhs,
                            start=(i == kept[0]), stop=(i == kept[-1]))
                    if s == NHS - 1:
                        emit_t7.lastpsum[half] = t7p
                        emit_t7.blocks[b] = t7sb
                        continue
                    nc.scalar.copy(
                        out=t7sb[:, sub * HS + 4 * half:
                                 sub * HS + 4 * (half + 1), :].rearrange(
                            "p a b -> p (a b)"),
                        in_=t7p)
                if sub == 1:
                    emit_t7.blocks[b] = t7sb

            emit_t7.blocks = {}
            emit_t7.lastpsum = [None, None]

            def emit_tap_muls(b, mts):
                """ACT-side scaled copies for the offloaded terms of block b.
                Emitted right after the mstrip's maxes so they never sit in
                front of the PSUM drains in the ACT queue."""
                ro = b * OS
                ms0 = (ro // MS) * MS
                tmps = []
                for i in OFFLOAD_TERMS:
                    _oh, _ow, d, dr, du = TERMS[i]
                    mt = mts[d]
                    j0, _j1 = JR[d]
                    base, _rows = MROW[d]
                    rr0 = ro - ms0 + 1 + dr - base
                    jj0 = du + 3 - j0
                    tmp = tappool.tile([P, OS, W], fp16, tag="tapmul")
                    nc.scalar.mul(out=tmp,
                                  in_=mt[:, rr0:rr0 + OS, jj0:jj0 + W],
                                  mul=w7f_s[:, i:i + 1])
                    tmps.append(tmp)
                return tmps

            def emit_tap_adds(b, tmps):
                """DVE accumulation of the offloaded-term products into the
                drained t7sb of block b."""
                t7sb = emit_t7.blocks[b]
                for tmp in tmps:
                    nc.vector.tensor_tensor(t7sb, t7sb, tmp, AOT.add)

            def emit_block_finish(b, split=False):
                """Patch wrap column, final multiply, output DMA for block b.

                With split=True the multiply+DMA run per 8-row half so the
                final strip's post-matmul chain is as short as possible.
                """
                ro = b * OS
                t7sb = emit_t7.blocks.pop(b)
                t3sb = emit_t3.blocks.pop(b)
                outs = opool.tile([P, OS, W], fp16, tag="outs")
                if not split:
                    nc.gpsimd.tensor_copy(t7sb[:, :, 0], t7c0[:, ro:ro + OS])
                    nc.vector.tensor_tensor(outs, t7sb, t3sb, AOT.mult)
                    nc.sync.dma_start(out=out_d.ap()[:, ro:ro + OS, :],
                                      in_=outs)
                else:
                    # first half: normal drained path, with the multiply
                    # on the Pool engine (idle at the tail) so its output
                    # DMA clears the wire before the PSUM-direct quarters
                    r0, r1 = 0, HS
                    nc.gpsimd.tensor_copy(t7sb[:, r0:r1, 0],
                                          t7c0[:, ro + r0:ro + r1])
                    nc.gpsimd.tensor_tensor(
                        outs[:, r0:r1, :], t7sb[:, r0:r1, :],
                        t3sb[:, r0:r1, :], AOT.mult)
                    nc.scalar.dma_start(
                        out=out_d.ap()[:, ro + r0:ro + r1, :],
                        in_=outs[:, r0:r1, :])
                    # second half: multiply straight from the PSUM
                    # accumulator in 4-row quarters so each output DMA can
                    # start as soon as its quarter is ready (shorter tail)
                    for qi, (q0, q1) in enumerate(((HS, HS + 4),
                                                   (HS + 4, OS))):
                        nc.vector.tensor_tensor(
                            outs[:, q0:q1, :].rearrange("p a b -> p (a b)"),
                            emit_t7.lastpsum[(q0 - HS) // 4],
                            t3sb[:, q0:q1, :].rearrange("p a b -> p (a b)"),
                            AOT.mult)
                        nc.vector.tensor_tensor(
                            outs[:, q0:q1, 0], t7c0[:, ro + q0:ro + q1],
                            t3sb[:, q0:q1, 0], AOT.mult)
                        # alternate issue queues so the two quarters' DMA
                        # setup overlaps at the very end of the kernel
                        eng = nc.scalar if qi == 0 else nc.sync
                        eng.dma_start(
                            out=out_d.ap()[:, ro + q0:ro + q1, :],
                            in_=outs[:, q0:q1, :])

            def emit_fixup():
                """t7c0[p, h] = sum_i w_i * max(A_i, B_i) at w=0 (DVE; the
                Pool engine's ucode has no max/STT support)."""
                for i, (oh, ow, _d, _dr, _du) in enumerate(TERMS):
                    tmp = fixp.tile([P, H], fp16, tag="fixtmp")
                    a_ap = canvas[:, 1:1 + H, 2 * oh + 7]
                    b_ap = canvas[:, oh:oh + H, (W + 6) + 2 * ow]
                    nc.vector.tensor_tensor(tmp, a_ap, b_ap, AOT.max)
                    nc.vector.scalar_tensor_tensor(
                        out=t7c0, in0=tmp, scalar=w7c_s[:, i:i + 1],
                        in1=(tmp if i == 0 else t7c0),
                        op0=AOT.mult, op1=(AOT.bypass if i == 0 else AOT.add))

            # PE warmup: tiny matmuls on the weight tiles establish the
            # p-state busy run before the real burst is dispatched.
            if NWARM:
                wm = ppool.tile([P, HS * W], fp32, tag="t3p")
                for wi in range(NWARM):
                    nc.tensor.matmul(
                        out=wm[:, 0:WROWS],
                        lhsT=wtile,
                        rhs=wtile[:, 0:WROWS],
                        start=True, stop=True)

            # ---- emission schedule ----
            # (the Tile scheduler reorders by dataflow; what matters here is
            # pool-buffer rotation: t3 four blocks ahead, maxes four mstrips
            # ahead of their consuming t7 matmuls)
            allm = {}
            for s in range(0, 8):                  # PE: t3 blocks 0-3
                emit_t3(s)
            for m in range(4):
                allm[m] = emit_maxes(m, split=(m == 0))
            emit_fixup()                           # DVE; waits on last chunk
            for m in range(NMS):                   # block b == mstrip m
                offl = m in OFFLOAD_MSTRIPS
                for s in (2 * m, 2 * m + 1):       # PE: t7 mstrip m
                    emit_t7(s, allm[m], skip=OFFLOAD_TERMS if offl else ())
                if offl:
                    # muls AFTER the strips: the PSUM drains must sit ahead
                    # of them in the in-order ACT queue (drains gate the PE
                    # PSUM rotation; products only gate the DVE adds)
                    tapt = emit_tap_muls(m, allm[m])
                    emit_tap_adds(m, tapt)
                emit_block_finish(m, split=(m == NMS - 1))
                if m + 4 < NMS:
                    emit_t3(2 * (m + 4))
                    emit_t3(2 * (m + 4) + 1)
                    allm[m + 4] = emit_maxes(m + 4)

    nc.compile()
    return nc


_PROGRAM = None


def _get_program():
    global _PROGRAM
    if _PROGRAM is None:
        _PROGRAM = build_program()
    return _PROGRAM


def make_in_maps(inputs):
    canv = build_host_canvas(inputs["x"])
    wt3, wt7, w7c = build_host_weights(inputs["w3"], inputs["w7"])
    in_maps = []
    for core in range(NCORES):
        in_maps.append({"canv": np.ascontiguousarray(canv[core]),
                        "wt3": wt3, "wt7": wt7, "w7c": w7c,
                        "w7f": w7c.astype(np.float32)})
    return in_maps


def kernel(**inputs) -> np.ndarray:
    from concourse.bass_utils import run_bass_kernel_spmd
    nc = _get_program()
    in_maps = make_in_maps(inputs)
    res = run_bass_kernel_spmd(nc, in_maps, core_ids=list(range(NCORES)))
    out = np.empty((N, C, H, W), dtype=np.float32)
    for core in range(NCORES):
        out[core * NLOC:(core + 1) * NLOC] = res.results[core]["out"].astype(
            np.float32).reshape(NLOC, C, H, W)
    return out



# revision 45
# speedup vs baseline: 1.0027x; 1.0027x over previous
"""Trainium2 Bass kernel v2 for nn_Kernel_12281761695451725822_53472342835843.

Computation (per sample n, channel c):
  t3 = Conv1x5(x, w3)                      (channel-mixing 1x5 conv, pad 2)
  t7 = sum over 9 (oh,ow) terms of w7[c,3*ow+oh] * max(A, B)
  out = t7 * t3

Multi-engine split (TimelineSim-balanced): PE does the t3 conv plus 7 of
the 9 t7 taps (diagonal matmuls, PSUM-accumulated); two taps are offloaded
as ACT per-partition scaled copies + DVE adds; DVE computes the 6 shared
pair-max tensors per 16-row strip (reading odd canvas offsets directly --
no shifted canvas copy) and the final t7*t3 multiply; ACT drains PSUM; the
gpsimd engine issues the casting input DMA (strided fp32->fp16 straight
into the padded canvas), zeroes borders and patches the w=0 wrap column.
Output is written fp16 and upcast on the host. PE warmup matmuls establish
the p-state ramp before the main burst; the input is chunked so the max
pipeline starts early and stays 4 strips ahead.
"""

import numpy as np

N, C, H, W = 16, 64, 128, 128
NCORES = 8
NLOC = N // NCORES          # samples per core
P = 128                     # partitions = NLOC * C
ROWS = H + 2                # canvas rows, storage row = h + 1
COLS = 140                  # canvas cols, storage col = u + UOFF, u in [-9, 131)
UOFF = 9
MS = 16                     # max-strip height (rows per M tile batch)
HS = 8                      # matmul strip height
OS = 16                     # output block rows per DMA
NMS = H // MS
NHS = H // HS
NOB = H // OS

# term i: (oh, ow, (dh, dw) of shared max, read offset (dr, du) into M)
TERMS = [
    (0, 0, (1, 1), -1, -3),
    (0, 1, (1, -1), -1, -1),
    (0, 2, (1, -3), -1, 1),
    (1, 0, (0, 3), 0, -3),
    (1, 1, (0, 1), 0, -1),
    (1, 2, (0, 1), 0, 0),
    (2, 0, (1, -5), 0, 2),
    (2, 1, (1, -3), 0, 2),
    (2, 2, (1, -1), 0, 2),
]
DELTAS = [(1, 1), (1, -1), (1, -3), (0, 3), (0, 1), (1, -5)]
# computed j ranges per delta: M_d[rr, j-j0] = max(x[h, j-3], x[h+dh, j-3+dw])
JR = {
    (1, 1): (0, 128),
    (1, -1): (2, 133),
    (1, -3): (4, 133),
    (0, 3): (0, 128),
    (0, 1): (2, 131),
    (1, -5): (5, 133),
}
# per delta: (row base, row count) of the stored M strip. rr index runs over
# [base, base+rows) where rr = h - ms0 + 1 + dr and dr is per consuming term.
MROW = {
    (1, 1): (0, MS),        # dr = -1 only
    (1, -1): (0, MS + 1),   # dr in {-1, 0}
    (1, -3): (0, MS + 1),
    (0, 3): (1, MS),        # dr = 0 only
    (0, 1): (1, MS),
    (1, -5): (1, MS),
}
# input DMA chunk boundaries in x-row space. First chunk is small so the
# first t3 matmuls can start early; maxes of mstrip m need x rows up to
# ms0+32 (canvas rows [ms0, ms0+33]).
CHUNKS = [(0, 5), (5, 9), (9, 17), (17, 34), (34, 50), (50, 66), (66, 82),
          (82, 98), (98, 114), (114, 128)]
NWARM = 30                  # PE warmup matmuls (p-state ramp establishment)
WROWS = 128                 # rows per warmup matmul
OFFLOAD_TERMS = (0, 3)      # t7 terms computed on ACT+DVE instead of PE
OFFLOAD_MSTRIPS = tuple(range(7))  # mstrips with offloaded terms (last stays
                                   # on PE to keep the tail chain short)


def build_host_weights(w3, w7):
    """Host-side packing of the conv weights into PE lhsT layouts (fp16)."""
    w3 = np.asarray(w3, dtype=np.float32)
    w7 = np.asarray(w7, dtype=np.float32)
    wt3 = np.zeros((5, P, P), dtype=np.float16)
    for k in range(5):
        blk = w3[:, :, 0, k].T.astype(np.float16)  # [ci, co]
        for n in range(NLOC):
            wt3[k, n * C:(n + 1) * C, n * C:(n + 1) * C] = blk
    wt7 = np.zeros((9, P, P), dtype=np.float16)
    w7c = np.zeros((P, 9), dtype=np.float16)
    for i, (oh, ow, _d, _dr, _du) in enumerate(TERMS):
        kidx = 3 * ow + oh
        vals = np.concatenate([w7[:, kidx], w7[:, kidx]]).astype(np.float16)  # [P]
        wt7[i, np.arange(P), np.arange(P)] = vals
        w7c[:, i] = vals
    return (np.ascontiguousarray(wt3.transpose(1, 0, 2)),
            np.ascontiguousarray(wt7.transpose(1, 0, 2)), w7c)


def build_host_canvas(x):
    """Pad + cast the input into the canvas layout on the host."""
    xs = np.asarray(x, dtype=np.float32).reshape(N, C, H, W)
    canv = np.zeros((NCORES, P, ROWS, COLS), dtype=np.float16)
    for core in range(NCORES):
        shard = xs[core * NLOC:(core + 1) * NLOC].reshape(P, H, W)
        canv[core, :, 1:H + 1, UOFF:UOFF + W] = shard.astype(np.float16)
    return canv


def build_program():
    """Build and compile the single-core Bass/Tile program (SPMD-replicated)."""
    import concourse.bacc as bacc
    import concourse.tile as tile
    import concourse.mybir as mybir

    fp16 = mybir.dt.float16
    fp32 = mybir.dt.float32
    AOT = mybir.AluOpType

    nc = bacc.Bacc("TRN2", target_bir_lowering=False, debug=False,
                   enable_asserts=False, num_devices=1)
    canv_d = nc.dram_tensor("canv", [P, ROWS, COLS], fp16, kind="ExternalInput")
    wt3_d = nc.dram_tensor("wt3", [P, 5, P], fp16, kind="ExternalInput")
    wt7_d = nc.dram_tensor("wt7", [P, 9, P], fp16, kind="ExternalInput")
    w7c_d = nc.dram_tensor("w7c", [P, 9], fp16, kind="ExternalInput")
    w7f_d = nc.dram_tensor("w7f", [P, 9], fp32, kind="ExternalInput")
    out_d = nc.dram_tensor("out", [P, H, W], fp16, kind="ExternalOutput")

    with tile.TileContext(nc) as tc:
        with (
            tc.tile_pool(name="persist", bufs=1) as persist,
            tc.tile_pool(name="mpool", bufs=4) as mpool,
            tc.tile_pool(name="ppool", bufs=2, space="PSUM") as ppool,
            tc.tile_pool(name="ppool7", bufs=4, space="PSUM") as ppool7,
            tc.tile_pool(name="s3pool", bufs=4) as s3pool,
            tc.tile_pool(name="s7pool", bufs=3) as s7pool,
            tc.tile_pool(name="opool", bufs=3) as opool,
            tc.tile_pool(name="fixp", bufs=2) as fixp,
            tc.tile_pool(name="tappool", bufs=4) as tappool,
        ):
            canvas = persist.tile([P, ROWS, COLS], fp16, tag="canvas")
            wtile = persist.tile([P, P], fp16, tag="warm")
            wt3_s = persist.tile([P, 5, P], fp16, tag="wt3")
            wt7_s = persist.tile([P, 9, P], fp16, tag="wt7")
            w7c_s = persist.tile([P, 9], fp16, tag="w7c")
            w7f_s = persist.tile([P, 9], fp32, tag="w7f")
            t7c0 = persist.tile([P, H], fp16, tag="t7c0")

            # warmup source: ready at ~0.3us, no DMA dependency
            nc.vector.memset(wtile, 0.25)

            # single SP HWDGE queue, explicit issue order: the two chunks
            # gating t3 strip 0 / first maxes, then weights, then fine-
            # grained bulk chunks (short DMA_ENGINES holds so out-DMAs
            # interleave without queueing behind long input transfers)
            def chunk_dma(ci):
                r0, r1 = CHUNKS[ci]
                c0 = 0 if ci == 0 else 1 + r0
                c1 = ROWS if ci == len(CHUNKS) - 1 else 1 + r1
                nc.sync.dma_start(out=canvas[:, c0:c1, :],
                                  in_=canv_d.ap()[:, c0:c1, :])

            chunk_dma(0)
            chunk_dma(1)
            nc.sync.dma_start(out=wt3_s, in_=wt3_d.ap())
            nc.sync.dma_start(out=wt7_s, in_=wt7_d.ap())
            nc.sync.dma_start(out=w7c_s, in_=w7c_d.ap())
            nc.sync.dma_start(out=w7f_s, in_=w7f_d.ap())
            for ci in range(2, len(CHUNKS)):
                chunk_dma(ci)

            def emit_maxes(m, split=False):
                """split=True: emit each max as two row-halves so the first
                half only depends on the first input chunk (earlier DVE
                start for mstrip 0)."""
                ms0 = m * MS
                mts = {}
                for di, (dh, dw) in enumerate(DELTAS):
                    j0, j1 = JR[(dh, dw)]
                    wd = j1 - j0
                    wpad = wd + (wd & 1)
                    base, rows = MROW[(dh, dw)]
                    mt = mpool.tile([P, rows, wpad], fp16, tag=f"m{di}")
                    pieces = [(0, rows)]
                    if split:
                        c1, c2 = 6 - base - dh, 10 - base - dh
                        pieces = [(0, c1), (c1, c2), (c2, rows)]
                    for (p0, p1) in pieces:
                        in0 = canvas[:, ms0 + base + p0:ms0 + base + p1,
                                     j0 + 6:j0 + 6 + wd]
                        in1 = canvas[:, ms0 + base + dh + p0:
                                     ms0 + base + dh + p1,
                                     j0 + 6 + dw:j0 + 6 + dw + wd]
                        nc.vector.tensor_tensor(mt[:, p0:p1, 0:wd],
                                                in0, in1, AOT.max)
                    mts[(dh, dw)] = mt
                return mts

            def emit_t3(s):
                """t3 matmuls for hstrip s, drain into t3sb block slice."""
                h0 = s * HS
                t3p = ppool.tile([P, HS * W], fp32, tag="t3p")
                for k in range(5):
                    for half in range(2):
                        rhs = canvas[:, 1 + h0 + 4 * half:1 + h0 + 4 * half + 4,
                                     k + 7:k + 7 + W]
                        nc.tensor.matmul(
                            out=t3p[:, 512 * half:512 * half + 512],
                            lhsT=wt3_s[:, k, :], rhs=rhs,
                            start=(k == 0), stop=(k == 4))
                b = h0 // OS
                sub = (h0 % OS) // HS
                if sub == 0:
                    t3sb = s3pool.tile([P, OS, W], fp16, tag="t3sb")
                    emit_t3.cur = t3sb
                t3sb = emit_t3.cur
                nc.scalar.copy(
                    out=t3sb[:, sub * HS:(sub + 1) * HS, :].rearrange(
                        "p a b -> p (a b)"),
                    in_=t3p)
                if sub == 1:
                    emit_t3.blocks[b] = t3sb

            emit_t3.blocks = {}

            def emit_t7(s, mts, skip=()):
                h0 = s * HS
                ms0 = (h0 // MS) * MS
                kept = [i for i in range(len(TERMS)) if i not in skip]
                b = h0 // OS
                sub = (h0 % OS) // HS
                if sub == 0:
                    t7sb = s7pool.tile([P, OS, W], fp16, tag="t7sb")
                    emit_t7.cur = t7sb
                t7sb = emit_t7.cur
                # single-bank PSUM tiles per 4-row half: 4-deep rotation
                # decouples the PE from ACT drain latency
                for half in range(2):
                    t7p = ppool7.tile([P, 512], fp32, tag="t7p")
                    for i, (_oh, _ow, d, dr, du) in enumerate(TERMS):
                        if i in skip:
                            continue
                        mt = mts[d]
                        j0, _j1 = JR[d]
                        base, _rows = MROW[d]
                        rr0 = h0 - ms0 + 1 + dr - base
                        jj0 = du + 3 - j0
                        rhs = mt[:, rr0 + 4 * half:rr0 + 4 * half + 4,
                                 jj0:jj0 + W]
                        nc.tensor.matmul(
                            out=t7p,
                            lhsT=wt7_s[:, i, :], rhs=rhs,
                            start=(i == kept[0]), stop=(i == kept[-1]))
                    if s == NHS - 1:
                        emit_t7.lastpsum[half] = t7p
                        emit_t7.blocks[b] = t7sb
                        continue
                    nc.scalar.copy(
                        out=t7sb[:, sub * HS + 4 * half:
                                 sub * HS + 4 * (half + 1), :].rearrange(
                            "p a b -> p (a b)"),
                        in_=t7p)
                if sub == 1:
                    emit_t7.blocks[b] = t7sb

            emit_t7.blocks = {}
            emit_t7.lastpsum = [None, None]

            def emit_tap_muls(b, mts):
                """ACT-side scaled copies for the offloaded terms of block b.
                Emitted right after the mstrip's maxes so they never sit in
                front of the PSUM drains in the ACT queue."""
                ro = b * OS
                ms0 = (ro // MS) * MS
                tmps = []
                for i in OFFLOAD_TERMS:
                    _oh, _ow, d, dr, du = TERMS[i]
                    mt = mts[d]
                    j0, _j1 = JR[d]
                    base, _rows = MROW[d]
                    rr0 = ro - ms0 + 1 + dr - base
                    jj0 = du + 3 - j0
                    tmp = tappool.tile([P, OS, W], fp16, tag="tapmul")
                    nc.scalar.mul(out=tmp,
                                  in_=mt[:, rr0:rr0 + OS, jj0:jj0 + W],
                                  mul=w7f_s[:, i:i + 1])
                    tmps.append(tmp)
                return tmps

            def emit_tap_adds(b, tmps):
                """DVE accumulation of the offloaded-term products into the
                drained t7sb of block b."""
                t7sb = emit_t7.blocks[b]
                for tmp in tmps:
                    nc.vector.tensor_tensor(t7sb, t7sb, tmp, AOT.add)

            def emit_block_finish(b, split=False):
                """Patch wrap column, final multiply, output DMA for block b.

                With split=True the multiply+DMA run per 8-row half so the
                final strip's post-matmul chain is as short as possible.
                """
                ro = b * OS
                t7sb = emit_t7.blocks.pop(b)
                t3sb = emit_t3.blocks.pop(b)
                outs = opool.tile([P, OS, W], fp16, tag="outs")
                if not split:
                    nc.gpsimd.tensor_copy(t7sb[:, :, 0], t7c0[:, ro:ro + OS])
                    nc.vector.tensor_tensor(outs, t7sb, t3sb, AOT.mult)
                    nc.sync.dma_start(out=out_d.ap()[:, ro:ro + OS, :],
                                      in_=outs)
                else:
                    # first half: normal drained path, with the multiply
                    # on the Pool engine (idle at the tail) so its output
                    # DMA clears the wire before the PSUM-direct quarters
                    r0, r1 = 0, HS
                    nc.gpsimd.tensor_copy(t7sb[:, r0:r1, 0],
                                          t7c0[:, ro + r0:ro + r1])
                    nc.gpsimd.tensor_tensor(
                        outs[:, r0:r1, :], t7sb[:, r0:r1, :],
                        t3sb[:, r0:r1, :], AOT.mult)
                    nc.scalar.dma_start(
                        out=out_d.ap()[:, ro + r0:ro + r1, :],
                        in_=outs[:, r0:r1, :])
                    # second half: multiply straight from the PSUM
                    # accumulator in 4-row quarters so each output DMA can
                    # start as soon as its quarter is ready (shorter tail)
                    for qi, (q0, q1) in enumerate(((HS, HS + 4),
                                                   (HS + 4, OS))):
                        nc.vector.tensor_tensor(
                            outs[:, q0:q1, :].rearrange("p a b -> p (a b)"),
                            emit_t7.lastpsum[(q0 - HS) // 4],
                            t3sb[:, q0:q1, :].rearrange("p a b -> p (a b)"),
                            AOT.mult)
                        nc.vector.tensor_tensor(
                            outs[:, q0:q1, 0], t7c0[:, ro + q0:ro + q1],
                            t3sb[:, q0:q1, 0], AOT.mult)
                        # alternate issue queues so the two quarters' DMA
                        # setup overlaps at the very end of the kernel
                        eng = nc.scalar if qi == 0 else nc.sync
                        eng.dma_start(
                            out=out_d.ap()[:, ro + q0:ro + q1, :],
                            in_=outs[:, q0:q1, :])

            def emit_fixup():
                """t7c0[p, h] = sum_i w_i * max(A_i, B_i) at w=0 (DVE; the
                Pool engine's ucode has no max/STT support)."""
                for i, (oh, ow, _d, _dr, _du) in enumerate(TERMS):
                    tmp = fixp.tile([P, H], fp16, tag="fixtmp")
                    a_ap = canvas[:, 1:1 + H, 2 * oh + 7]
                    b_ap = canvas[:, oh:oh + H, (W + 6) + 2 * ow]
                    nc.vector.tensor_tensor(tmp, a_ap, b_ap, AOT.max)
                    nc.vector.scalar_tensor_tensor(
                        out=t7c0, in0=tmp, scalar=w7c_s[:, i:i + 1],
                        in1=(tmp if i == 0 else t7c0),
                        op0=AOT.mult, op1=(AOT.bypass if i == 0 else AOT.add))

            # PE warmup: tiny matmuls on the weight tiles establish the
            # p-state busy run before the real burst is dispatched.
            if NWARM:
                wm = ppool.tile([P, HS * W], fp32, tag="t3p")
                for wi in range(NWARM):
                    nc.tensor.matmul(
                        out=wm[:, 0:WROWS],
                        lhsT=wtile,
                        rhs=wtile[:, 0:WROWS],
                        start=True, stop=True)

            # ---- emission schedule ----
            # (the Tile scheduler reorders by dataflow; what matters here is
            # pool-buffer rotation: t3 four blocks ahead, maxes four mstrips
            # ahead of their consuming t7 matmuls)
            allm = {}
            for s in range(0, 8):                  # PE: t3 blocks 0-3
                emit_t3(s)
            for m in range(4):
                allm[m] = emit_maxes(m, split=(m == 0))
            emit_fixup()                           # DVE; waits on last chunk
            for m in range(NMS):                   # block b == mstrip m
                offl = m in OFFLOAD_MSTRIPS
                for s in (2 * m, 2 * m + 1):       # PE: t7 mstrip m
                    emit_t7(s, allm[m], skip=OFFLOAD_TERMS if offl else ())
                if offl:
                    # muls AFTER the strips: the PSUM drains must sit ahead
                    # of them in the in-order ACT queue (drains gate the PE
                    # PSUM rotation; products only gate the DVE adds)
                    tapt = emit_tap_muls(m, allm[m])
                    emit_tap_adds(m, tapt)
                emit_block_finish(m, split=(m == NMS - 1))
                if m + 4 < NMS:
                    emit_t3(2 * (m + 4))
                    emit_t3(2 * (m + 4) + 1)
                    allm[m + 4] = emit_maxes(m + 4)

    nc.compile()
    return nc


_PROGRAM = None


def _get_program():
    global _PROGRAM
    if _PROGRAM is None:
        _PROGRAM = build_program()
    return _PROGRAM


def make_in_maps(inputs):
    canv = build_host_canvas(inputs["x"])
    wt3, wt7, w7c = build_host_weights(inputs["w3"], inputs["w7"])
    in_maps = []
    for core in range(NCORES):
        in_maps.append({"canv": np.ascontiguousarray(canv[core]),
                        "wt3": wt3, "wt7": wt7, "w7c": w7c,
                        "w7f": w7c.astype(np.float32)})
    return in_maps


def kernel(**inputs) -> np.ndarray:
    from concourse.bass_utils import run_bass_kernel_spmd
    nc = _get_program()
    in_maps = make_in_maps(inputs)
    res = run_bass_kernel_spmd(nc, in_maps, core_ids=list(range(NCORES)))
    out = np.empty((N, C, H, W), dtype=np.float32)
    for core in range(NCORES):
        out[core * NLOC:(core + 1) * NLOC] = res.results[core]["out"].astype(
            np.float32).reshape(NLOC, C, H, W)
    return out



# revision 48
# speedup vs baseline: 1.0059x; 1.0032x over previous
"""Trainium2 Bass kernel v2 for nn_Kernel_12281761695451725822_53472342835843.

Computation (per sample n, channel c):
  t3 = Conv1x5(x, w3)                      (channel-mixing 1x5 conv, pad 2)
  t7 = sum over 9 (oh,ow) terms of w7[c,3*ow+oh] * max(A, B)
  out = t7 * t3

Multi-engine split (TimelineSim-balanced): PE does the t3 conv plus 7 of
the 9 t7 taps (diagonal matmuls, PSUM-accumulated); two taps are offloaded
as ACT per-partition scaled copies + DVE adds; DVE computes the 6 shared
pair-max tensors per 16-row strip (reading odd canvas offsets directly --
no shifted canvas copy) and the final t7*t3 multiply; ACT drains PSUM; the
gpsimd engine issues the casting input DMA (strided fp32->fp16 straight
into the padded canvas), zeroes borders and patches the w=0 wrap column.
Output is written fp16 and upcast on the host. PE warmup matmuls establish
the p-state ramp before the main burst; the input is chunked so the max
pipeline starts early and stays 4 strips ahead.
"""

import numpy as np

N, C, H, W = 16, 64, 128, 128
NCORES = 8
NLOC = N // NCORES          # samples per core
P = 128                     # partitions = NLOC * C
ROWS = H + 2                # canvas rows, storage row = h + 1
COLS = 140                  # canvas cols, storage col = u + UOFF, u in [-9, 131)
UOFF = 9
MS = 16                     # max-strip height (rows per M tile batch)
HS = 8                      # matmul strip height
OS = 16                     # output block rows per DMA
NMS = H // MS
NHS = H // HS
NOB = H // OS

# term i: (oh, ow, (dh, dw) of shared max, read offset (dr, du) into M)
TERMS = [
    (0, 0, (1, 1), -1, -3),
    (0, 1, (1, -1), -1, -1),
    (0, 2, (1, -3), -1, 1),
    (1, 0, (0, 3), 0, -3),
    (1, 1, (0, 1), 0, -1),
    (1, 2, (0, 1), 0, 0),
    (2, 0, (1, -5), 0, 2),
    (2, 1, (1, -3), 0, 2),
    (2, 2, (1, -1), 0, 2),
]
DELTAS = [(1, 1), (1, -1), (1, -3), (0, 3), (0, 1), (1, -5)]
# computed j ranges per delta: M_d[rr, j-j0] = max(x[h, j-3], x[h+dh, j-3+dw])
JR = {
    (1, 1): (0, 128),
    (1, -1): (2, 133),
    (1, -3): (4, 133),
    (0, 3): (0, 128),
    (0, 1): (2, 131),
    (1, -5): (5, 133),
}
# per delta: (row base, row count) of the stored M strip. rr index runs over
# [base, base+rows) where rr = h - ms0 + 1 + dr and dr is per consuming term.
MROW = {
    (1, 1): (0, MS),        # dr = -1 only
    (1, -1): (0, MS + 1),   # dr in {-1, 0}
    (1, -3): (0, MS + 1),
    (0, 3): (1, MS),        # dr = 0 only
    (0, 1): (1, MS),
    (1, -5): (1, MS),
}
# input DMA chunk boundaries in x-row space. First chunk is small so the
# first t3 matmuls can start early; maxes of mstrip m need x rows up to
# ms0+32 (canvas rows [ms0, ms0+33]).
CHUNKS = [(0, 5), (5, 9), (9, 17), (17, 34), (34, 50), (50, 66), (66, 82),
          (82, 98), (98, 114), (114, 128)]
NWARM = 30                  # PE warmup matmuls (p-state ramp establishment)
WROWS = 128                 # rows per warmup matmul
OFFLOAD_TERMS = (0, 3)      # t7 terms computed on ACT+DVE instead of PE
OFFLOAD_MSTRIPS = tuple(range(7))  # mstrips with offloaded terms (last stays
                                   # on PE to keep the tail chain short)


def build_host_weights(w3, w7):
    """Host-side packing of the conv weights into PE lhsT layouts (fp16)."""
    w3 = np.asarray(w3, dtype=np.float32)
    w7 = np.asarray(w7, dtype=np.float32)
    wt3 = np.zeros((5, P, P), dtype=np.float16)
    for k in range(5):
        blk = w3[:, :, 0, k].T.astype(np.float16)  # [ci, co]
        for n in range(NLOC):
            wt3[k, n * C:(n + 1) * C, n * C:(n + 1) * C] = blk
    wt7 = np.zeros((9, P, P), dtype=np.float16)
    w7c = np.zeros((P, 9), dtype=np.float16)
    for i, (oh, ow, _d, _dr, _du) in enumerate(TERMS):
        kidx = 3 * ow + oh
        vals = np.concatenate([w7[:, kidx], w7[:, kidx]]).astype(np.float16)  # [P]
        wt7[i, np.arange(P), np.arange(P)] = vals
        w7c[:, i] = vals
    return (np.ascontiguousarray(wt3.transpose(1, 0, 2)),
            np.ascontiguousarray(wt7.transpose(1, 0, 2)), w7c)


def build_host_canvas(x):
    """Pad + cast the input into the canvas layout on the host."""
    xs = np.asarray(x, dtype=np.float32).reshape(N, C, H, W)
    canv = np.zeros((NCORES, P, ROWS, COLS), dtype=np.float16)
    for core in range(NCORES):
        shard = xs[core * NLOC:(core + 1) * NLOC].reshape(P, H, W)
        canv[core, :, 1:H + 1, UOFF:UOFF + W] = shard.astype(np.float16)
    return canv


def build_program():
    """Build and compile the single-core Bass/Tile program (SPMD-replicated)."""
    import concourse.bacc as bacc
    import concourse.tile as tile
    import concourse.mybir as mybir

    fp16 = mybir.dt.float16
    fp32 = mybir.dt.float32
    AOT = mybir.AluOpType

    nc = bacc.Bacc("TRN2", target_bir_lowering=False, debug=False,
                   enable_asserts=False, num_devices=1)
    canv_d = nc.dram_tensor("canv", [P, ROWS, COLS], fp16, kind="ExternalInput")
    wt3_d = nc.dram_tensor("wt3", [P, 5, P], fp16, kind="ExternalInput")
    wt7_d = nc.dram_tensor("wt7", [P, 9, P], fp16, kind="ExternalInput")
    w7c_d = nc.dram_tensor("w7c", [P, 9], fp16, kind="ExternalInput")
    w7f_d = nc.dram_tensor("w7f", [P, 9], fp32, kind="ExternalInput")
    out_d = nc.dram_tensor("out", [P, H, W], fp16, kind="ExternalOutput")

    with tile.TileContext(nc) as tc:
        with (
            tc.tile_pool(name="persist", bufs=1) as persist,
            tc.tile_pool(name="mpool", bufs=4) as mpool,
            tc.tile_pool(name="ppool", bufs=2, space="PSUM") as ppool,
            tc.tile_pool(name="ppool7", bufs=4, space="PSUM") as ppool7,
            tc.tile_pool(name="s3pool", bufs=4) as s3pool,
            tc.tile_pool(name="s7pool", bufs=3) as s7pool,
            tc.tile_pool(name="opool", bufs=3) as opool,
            tc.tile_pool(name="fixp", bufs=2) as fixp,
            tc.tile_pool(name="tappool", bufs=4) as tappool,
        ):
            canvas = persist.tile([P, ROWS, COLS], fp16, tag="canvas")
            wtile = persist.tile([P, P], fp16, tag="warm")
            wt3_s = persist.tile([P, 5, P], fp16, tag="wt3")
            wt7_s = persist.tile([P, 9, P], fp16, tag="wt7")
            w7c_s = persist.tile([P, 9], fp16, tag="w7c")
            w7f_s = persist.tile([P, 9], fp32, tag="w7f")
            t7c0 = persist.tile([P, H], fp16, tag="t7c0")

            # warmup source: ready at ~0.3us, no DMA dependency
            nc.vector.memset(wtile, 0.25)

            # single SP HWDGE queue, explicit issue order: the two chunks
            # gating t3 strip 0 / first maxes, then weights, then fine-
            # grained bulk chunks (short DMA_ENGINES holds so out-DMAs
            # interleave without queueing behind long input transfers)
            def chunk_dma(ci):
                r0, r1 = CHUNKS[ci]
                c0 = 0 if ci == 0 else 1 + r0
                c1 = ROWS if ci == len(CHUNKS) - 1 else 1 + r1
                nc.sync.dma_start(out=canvas[:, c0:c1, :],
                                  in_=canv_d.ap()[:, c0:c1, :])

            chunk_dma(0)
            chunk_dma(1)
            nc.sync.dma_start(out=wt3_s, in_=wt3_d.ap())
            nc.sync.dma_start(out=wt7_s, in_=wt7_d.ap())
            nc.sync.dma_start(out=w7c_s, in_=w7c_d.ap())
            nc.sync.dma_start(out=w7f_s, in_=w7f_d.ap())
            for ci in range(2, len(CHUNKS)):
                chunk_dma(ci)

            def emit_maxes(m, split=False):
                """split=True: emit each max as two row-halves so the first
                half only depends on the first input chunk (earlier DVE
                start for mstrip 0)."""
                ms0 = m * MS
                mts = {}
                for di, (dh, dw) in enumerate(DELTAS):
                    j0, j1 = JR[(dh, dw)]
                    wd = j1 - j0
                    wpad = wd + (wd & 1)
                    base, rows = MROW[(dh, dw)]
                    mt = mpool.tile([P, rows, wpad], fp16, tag=f"m{di}")
                    pieces = [(0, rows)]
                    if split:
                        c1, c2 = 6 - base - dh, 10 - base - dh
                        pieces = [(0, c1), (c1, c2), (c2, rows)]
                    for (p0, p1) in pieces:
                        in0 = canvas[:, ms0 + base + p0:ms0 + base + p1,
                                     j0 + 6:j0 + 6 + wd]
                        in1 = canvas[:, ms0 + base + dh + p0:
                                     ms0 + base + dh + p1,
                                     j0 + 6 + dw:j0 + 6 + dw + wd]
                        nc.vector.tensor_tensor(mt[:, p0:p1, 0:wd],
                                                in0, in1, AOT.max)
                    mts[(dh, dw)] = mt
                return mts

            def emit_t3(s):
                """t3 matmuls for hstrip s, drain into t3sb block slice."""
                h0 = s * HS
                t3p = ppool.tile([P, HS * W], fp32, tag="t3p")
                for k in range(5):
                    for half in range(2):
                        rhs = canvas[:, 1 + h0 + 4 * half:1 + h0 + 4 * half + 4,
                                     k + 7:k + 7 + W]
                        nc.tensor.matmul(
                            out=t3p[:, 512 * half:512 * half + 512],
                            lhsT=wt3_s[:, k, :], rhs=rhs,
                            start=(k == 0), stop=(k == 4))
                b = h0 // OS
                sub = (h0 % OS) // HS
                if sub == 0:
                    t3sb = s3pool.tile([P, OS, W], fp16, tag="t3sb")
                    emit_t3.cur = t3sb
                t3sb = emit_t3.cur
                nc.scalar.copy(
                    out=t3sb[:, sub * HS:(sub + 1) * HS, :].rearrange(
                        "p a b -> p (a b)"),
                    in_=t3p)
                if sub == 1:
                    emit_t3.blocks[b] = t3sb

            emit_t3.blocks = {}

            def emit_t7(s, mts, skip=()):
                h0 = s * HS
                ms0 = (h0 // MS) * MS
                kept = [i for i in range(len(TERMS)) if i not in skip]
                b = h0 // OS
                sub = (h0 % OS) // HS
                if sub == 0:
                    t7sb = s7pool.tile([P, OS, W], fp16, tag="t7sb")
                    emit_t7.cur = t7sb
                t7sb = emit_t7.cur
                # single-bank PSUM tiles per 4-row half: 4-deep rotation
                # decouples the PE from ACT drain latency
                for half in range(2):
                    t7p = ppool7.tile([P, 512], fp32, tag="t7p")
                    for i, (_oh, _ow, d, dr, du) in enumerate(TERMS):
                        if i in skip:
                            continue
                        mt = mts[d]
                        j0, _j1 = JR[d]
                        base, _rows = MROW[d]
                        rr0 = h0 - ms0 + 1 + dr - base
                        jj0 = du + 3 - j0
                        rhs = mt[:, rr0 + 4 * half:rr0 + 4 * half + 4,
                                 jj0:jj0 + W]
                        nc.tensor.matmul(
                            out=t7p,
                            lhsT=wt7_s[:, i, :], rhs=rhs,
                            start=(i == kept[0]), stop=(i == kept[-1]))
                    if s == NHS - 1:
                        emit_t7.lastpsum[half] = t7p
                        emit_t7.blocks[b] = t7sb
                        continue
                    nc.scalar.copy(
                        out=t7sb[:, sub * HS + 4 * half:
                                 sub * HS + 4 * (half + 1), :].rearrange(
                            "p a b -> p (a b)"),
                        in_=t7p)
                if sub == 1:
                    emit_t7.blocks[b] = t7sb

            emit_t7.blocks = {}
            emit_t7.lastpsum = [None, None]

            def emit_tap_muls(b, mts):
                """ACT-side scaled copies for the offloaded terms of block b.
                Emitted right after the mstrip's maxes so they never sit in
                front of the PSUM drains in the ACT queue."""
                ro = b * OS
                ms0 = (ro // MS) * MS
                tmps = []
                for i in OFFLOAD_TERMS:
                    _oh, _ow, d, dr, du = TERMS[i]
                    mt = mts[d]
                    j0, _j1 = JR[d]
                    base, _rows = MROW[d]
                    rr0 = ro - ms0 + 1 + dr - base
                    jj0 = du + 3 - j0
                    tmp = tappool.tile([P, OS, W], fp16, tag="tapmul")
                    nc.scalar.mul(out=tmp,
                                  in_=mt[:, rr0:rr0 + OS, jj0:jj0 + W],
                                  mul=w7f_s[:, i:i + 1])
                    tmps.append(tmp)
                return tmps

            def emit_tap_adds(b, tmps):
                """DVE accumulation of the offloaded-term products into the
                drained t7sb of block b."""
                t7sb = emit_t7.blocks[b]
                for tmp in tmps:
                    nc.vector.tensor_tensor(t7sb, t7sb, tmp, AOT.add)

            def emit_block_finish(b, split=False):
                """Patch wrap column, final multiply, output DMA for block b.

                With split=True the multiply+DMA run per 8-row half so the
                final strip's post-matmul chain is as short as possible.
                """
                ro = b * OS
                t7sb = emit_t7.blocks.pop(b)
                t3sb = emit_t3.blocks.pop(b)
                outs = opool.tile([P, OS, W], fp16, tag="outs")
                if not split:
                    nc.gpsimd.tensor_copy(t7sb[:, :, 0], t7c0[:, ro:ro + OS])
                    nc.vector.tensor_tensor(outs, t7sb, t3sb, AOT.mult)
                    nc.sync.dma_start(out=out_d.ap()[:, ro:ro + OS, :],
                                      in_=outs)
                else:
                    # first half: normal drained path, with the multiply
                    # on the Pool engine (idle at the tail) so its output
                    # DMA clears the wire before the PSUM-direct quarters
                    r0, r1 = 0, HS
                    nc.gpsimd.tensor_copy(t7sb[:, r0:r1, 0],
                                          t7c0[:, ro + r0:ro + r1])
                    nc.gpsimd.tensor_tensor(
                        outs[:, r0:r1, :], t7sb[:, r0:r1, :],
                        t3sb[:, r0:r1, :], AOT.mult)
                    nc.scalar.dma_start(
                        out=out_d.ap()[:, ro + r0:ro + r1, :],
                        in_=outs[:, r0:r1, :])
                    # second half: multiply straight from the PSUM
                    # accumulator in 4-row quarters so each output DMA can
                    # start as soon as its quarter is ready (shorter tail)
                    for qi, (q0, q1) in enumerate(((HS, HS + 4),
                                                   (HS + 4, OS))):
                        nc.vector.tensor_tensor(
                            outs[:, q0:q1, :].rearrange("p a b -> p (a b)"),
                            emit_t7.lastpsum[(q0 - HS) // 4],
                            t3sb[:, q0:q1, :].rearrange("p a b -> p (a b)"),
                            AOT.mult)
                        nc.vector.tensor_tensor(
                            outs[:, q0:q1, 0], t7c0[:, ro + q0:ro + q1],
                            t3sb[:, q0:q1, 0], AOT.mult)
                        # alternate issue queues so the two quarters' DMA
                        # setup overlaps at the very end of the kernel
                        eng = nc.scalar if qi == 0 else nc.sync
                        eng.dma_start(
                            out=out_d.ap()[:, ro + q0:ro + q1, :],
                            in_=outs[:, q0:q1, :])

            def emit_fixup():
                """t7c0[p, h] = sum_i w_i * max(A_i, B_i) at w=0 (DVE).

                The three ow-terms of each oh share the A operand, so the
                maxes group into one 3-wide op per oh (DVE-queue chain cut);
                accumulation order matches TERMS exactly."""
                for oh in range(3):
                    tmp = fixp.tile([P, 3, H], fp16, tag="fixtmp")
                    a3 = canvas[:, 1:1 + H, 2 * oh + 7].unsqueeze(1)\
                        .broadcast_to([P, 3, H])
                    b3 = canvas[:, oh:oh + H, W + 6:W + 12].rearrange(
                        "p h (a b) -> p h a b", b=2)[:, :, :, 0]\
                        .transpose([0, 2, 1])
                    nc.vector.tensor_tensor(tmp, a3, b3, AOT.max)
                    for ow in range(3):
                        i = 3 * oh + ow
                        nc.vector.scalar_tensor_tensor(
                            out=t7c0, in0=tmp[:, ow, :],
                            scalar=w7c_s[:, i:i + 1],
                            in1=(tmp[:, ow, :] if i == 0 else t7c0),
                            op0=AOT.mult,
                            op1=(AOT.bypass if i == 0 else AOT.add))

            # PE warmup: tiny matmuls on the weight tiles establish the
            # p-state busy run before the real burst is dispatched.
            if NWARM:
                wm = ppool.tile([P, HS * W], fp32, tag="t3p")
                for wi in range(NWARM):
                    nc.tensor.matmul(
                        out=wm[:, 0:WROWS],
                        lhsT=wtile,
                        rhs=wtile[:, 0:WROWS],
                        start=True, stop=True)

            # ---- emission schedule ----
            # (the Tile scheduler reorders by dataflow; what matters here is
            # pool-buffer rotation: t3 four blocks ahead, maxes four mstrips
            # ahead of their consuming t7 matmuls)
            allm = {}
            for s in range(0, 8):                  # PE: t3 blocks 0-3
                emit_t3(s)
            for m in range(4):
                allm[m] = emit_maxes(m, split=(m == 0))
            emit_fixup()                           # DVE; waits on last chunk
            for m in range(NMS):                   # block b == mstrip m
                offl = m in OFFLOAD_MSTRIPS
                for s in (2 * m, 2 * m + 1):       # PE: t7 mstrip m
                    emit_t7(s, allm[m], skip=OFFLOAD_TERMS if offl else ())
                if offl:
                    # muls AFTER the strips: the PSUM drains must sit ahead
                    # of them in the in-order ACT queue (drains gate the PE
                    # PSUM rotation; products only gate the DVE adds)
                    tapt = emit_tap_muls(m, allm[m])
                    emit_tap_adds(m, tapt)
                emit_block_finish(m, split=(m == NMS - 1))
                if m + 4 < NMS:
                    emit_t3(2 * (m + 4))
                    emit_t3(2 * (m + 4) + 1)
                    allm[m + 4] = emit_maxes(m + 4)

    nc.compile()
    return nc


_PROGRAM = None


def _get_program():
    global _PROGRAM
    if _PROGRAM is None:
        _PROGRAM = build_program()
    return _PROGRAM


def make_in_maps(inputs):
    canv = build_host_canvas(inputs["x"])
    wt3, wt7, w7c = build_host_weights(inputs["w3"], inputs["w7"])
    in_maps = []
    for core in range(NCORES):
        in_maps.append({"canv": np.ascontiguousarray(canv[core]),
                        "wt3": wt3, "wt7": wt7, "w7c": w7c,
                        "w7f": w7c.astype(np.float32)})
    return in_maps


def kernel(**inputs) -> np.ndarray:
    from concourse.bass_utils import run_bass_kernel_spmd
    nc = _get_program()
    in_maps = make_in_maps(inputs)
    res = run_bass_kernel_spmd(nc, in_maps, core_ids=list(range(NCORES)))
    out = np.empty((N, C, H, W), dtype=np.float32)
    for core in range(NCORES):
        out[core * NLOC:(core + 1) * NLOC] = res.results[core]["out"].astype(
            np.float32).reshape(NLOC, C, H, W)
    return out

